# revision 11
# baseline (speedup 1.0000x reference)
"""Trainium2 kernel for nn_ContrastiveLoss (N=4096, D=1024), SPMD over 8 NeuronCores.

Strategy (2x4 core grid, fp8 DoubleRow matmuls at the PE roofline):
  - Host: l2-normalize back_VF/back_AF in f64, scale by 16 and quantize to
    e4m3, pre-transpose into DoubleRow-blocked layouts, compute diag sims
    and the pre-feature cosine term (both O(N*D), same class as the
    normalization already done here).
  - Core (rg, cg) of a 2x4 grid computes its [2048, 1024] tile of
    E = exp(Vn @ An^T):
      * TensorE: 16 groups x 8 fp8 DoubleRow matmuls (K=256 each) into a
        [128, 1024] PSUM tile; short HAM-warmup matmuls first so the clock
        ramp overlaps the initial DMA wait
      * ScalarE: exp(PSUM / 256) with fused row-sum (accum_out); no DMA
        issues on ScalarE so PSUM drains at stream rate
      * VectorE: 15 f32 adds accumulate the column partials; last add emits
        bf16 for the output DMA
      * DMA: 7 big input dma_starts (an blocks 512KB, vn in 5 chunks)
        split across sync+scalar so the first matmul group's inputs land
        first; 2 small output DMAs on sync
    Outputs per core: rowsum partials [128, 16], column partials
    [128, 1024] bf16 (partition-summed on host).
  - Host: O(N) final assembly (log/ratio/sums) in f64.
"""

import os
import sys

import numpy as np

for _p in ("/opt/trn_rl_repo",):
    if _p not in sys.path and os.path.isdir(_p):
        sys.path.insert(0, _p)

N = 4096
D = 1024
NCORES = 8
RG = 2                   # row groups
CG = 4                   # col groups
ROWS = N // RG           # 2048 rows per core
COLS = N // CG           # 1024 cols per core
MCH = ROWS // 128        # 16 row chunks per core
KCH = D // 128           # 8 contraction chunks of 128
KD2 = KCH // 2           # 4 DoubleRow chunks of 256
NB = 512                 # matmul moving free dim (one PSUM half)
NBL = COLS // NB         # 2 column blocks per core

MARGIN = 0.2
BALANCE = 0.5
BIAS = 1.0
EPS = 1e-18

FP8_SCALE = 16.0  # host pre-scale so e4m3 keeps the values out of subnormals

_CACHE = {}
LAST_RESULT = None  # BassKernelResults of the most recent run (for test harness)


def _build_nc():
    import concourse.bass as bass  # noqa: F401
    import concourse.bacc as bacc
    import concourse.tile as tile
    from concourse import mybir
    from contextlib import ExitStack

    BF16 = mybir.dt.bfloat16
    F32 = mybir.dt.float32
    FP8 = mybir.dt.float8e4
    Exp = mybir.ActivationFunctionType.Exp
    DoubleRow = mybir.MatmulPerfMode.DoubleRow

    nc = bacc.Bacc("TRN2", debug=False, num_devices=NCORES)

    # DRAM I/O (per core). Layouts chosen so every DMA is contiguous.
    # vnT[p, mc*1024 + k2*256 + i*128 + dm] = Vn_slab[mc*128 + dm, (2*k2+i)*128 + p] * FP8_SCALE
    vnT_d = nc.dram_tensor("vnT", [128, MCH * KD2 * 2 * 128], FP8, kind="ExternalInput")
    # anT[n, p, k2*2*NB + i*NB + c] = An_slab[n*NB + c, (2*k2+i)*128 + p] * FP8_SCALE
    anT_d = nc.dram_tensor("anT", [NBL, 128, KCH * NB], FP8, kind="ExternalInput")

    # rowsum[p, mc] = sum over this core's 1024 cols of E_slab[mc*128 + p, :]
    # (last chunk split into two half-accums, host sums columns MCH-1 and MCH)
    rowsum_d = nc.dram_tensor("rowsum", [128, MCH + 1], F32, kind="ExternalOutput")
    # colp[p, j] = sum over mc of exp chunk [mc][p, j]  (host sums partitions)
    colp_d = nc.dram_tensor("colp", [128, COLS], BF16, kind="ExternalOutput")

    with tile.TileContext(nc) as tc:
        with ExitStack() as ctx:
            singles = ctx.enter_context(tc.tile_pool(name="singles", bufs=1))

            vn_sb = singles.tile([128, MCH * KD2 * 2 * 128], FP8, tag="vn")
            an_sb = []
            for n in range(NBL):
                an_t = singles.tile([128, KCH * NB], FP8, tag=f"an{n}")
                an_sb.append(an_t)

            # Input DMAs. Constraints learned from traces: the sync HWDGE
            # queue starts moving data ~1us earlier than scalar's; DMAs to
            # the SAME tile from different engines get serialized by
            # completion-waits (coarse WAW interval overlap), so each tile's
            # chunks stay on one engine. Issue in stream-consumption order;
            # late vn chunks last so they can't steal early bandwidth.
            MCW = 1024  # vnT columns per row chunk
            AKW = 1024  # anT columns per k2 slice
            nc.sync.dma_start(vn_sb[:, 0 : 2 * MCW], vnT_d.ap()[:, 0 : 2 * MCW])
            nc.scalar.dma_start(an_sb[0][:, 0:AKW], anT_d.ap()[0][:, 0:AKW])
            nc.sync.dma_start(an_sb[1][:, 0 : 2 * AKW], anT_d.ap()[1][:, 0 : 2 * AKW])
            nc.scalar.dma_start(
                an_sb[0][:, AKW : 2 * AKW], anT_d.ap()[0][:, AKW : 2 * AKW]
            )
            nc.sync.dma_start(
                an_sb[1][:, 2 * AKW : 4 * AKW], anT_d.ap()[1][:, 2 * AKW : 4 * AKW]
            )
            nc.scalar.dma_start(
                an_sb[0][:, 2 * AKW : 4 * AKW], anT_d.ap()[0][:, 2 * AKW : 4 * AKW]
            )
            nc.sync.dma_start(
                vn_sb[:, 2 * MCW : 4 * MCW], vnT_d.ap()[:, 2 * MCW : 4 * MCW]
            )
            nc.sync.dma_start(
                vn_sb[:, 4 * MCW : 8 * MCW], vnT_d.ap()[:, 4 * MCW : 8 * MCW]
            )
            nc.sync.dma_start(
                vn_sb[:, 8 * MCW : 12 * MCW], vnT_d.ap()[:, 8 * MCW : 12 * MCW]
            )
            nc.sync.dma_start(
                vn_sb[:, 12 * MCW : 16 * MCW], vnT_d.ap()[:, 12 * MCW : 16 * MCW]
            )

            efold = singles.tile([128, COLS], F32, tag="efold")
            colp = singles.tile([128, COLS], BF16, tag="colp")
            rs = singles.tile([128, MCH + 1], F32, tag="rs")
            ones_b = singles.tile([128, 1], BF16, tag="ones_b")
            nc.vector.memset(ones_b[:], 1.0)
            dummy = singles.tile([128, 256], BF16, tag="dummy")
            nc.vector.memset(dummy[:], 0.0)

            psum = ctx.enter_context(tc.tile_pool(name="mm_psum", bufs=3, space="PSUM"))
            wup = ctx.enter_context(tc.tile_pool(name="wup_psum", bufs=1, space="PSUM"))
            epool = ctx.enter_context(tc.tile_pool(name="etile", bufs=3))

            # HAM warmup: keep TensorE busy during the initial DMA wait so
            # the clock ramp starts as early as possible.
            wps = wup.tile([128, 256], mybir.dt.float32, tag="wup")
            NWARM = 12
            for i in range(NWARM):
                nc.tensor.matmul(
                    wps[0:1, :], ones_b[:], dummy[:],
                    start=(i == 0), stop=(i == NWARM - 1),
                )

            # Main stream: 16 groups of 8 DoubleRow matmuls. k2-outer,
            # half-inner matches the k-chunked DMA arrival order, so the
            # first group starts as soon as the first k-slices land.
            descale = 1.0 / (FP8_SCALE * FP8_SCALE)
            for mc in range(MCH):
                ps = psum.tile([128, 2 * NB], mybir.dt.float32)
                for k2 in range(KD2):
                    w3 = (
                        vn_sb[:, mc * MCW + k2 * 256 : mc * MCW + (k2 + 1) * 256]
                        .rearrange("p (i m) -> p i m", i=2)
                    )
                    for half in range(NBL):
                        a3 = (
                            an_sb[half][:, k2 * 2 * NB : (k2 + 1) * 2 * NB]
                            .rearrange("p (i c) -> p i c", i=2)
                        )
                        nc.tensor.matmul(
                            ps[:, half * NB : (half + 1) * NB],
                            w3,
                            a3,
                            start=(k2 == 0),
                            stop=(k2 == KD2 - 1),
                            perf_mode=DoubleRow,
                        )
                if mc == 0:
                    nc.scalar.activation(
                        efold[:], ps[:], Exp, scale=descale,
                        accum_out=rs[:, mc : mc + 1],
                    )
                elif mc == MCH - 1:
                    # Last chunk: halve the serial exp->add->DMA tail. Two
                    # [128, 512] exps (row-sum halves land in rs cols MCH-1
                    # and MCH; host adds), two bf16 adds, per-half output
                    # DMAs overlapped with the second half's compute.
                    et = epool.tile([128, 2 * NB], F32)
                    for h in range(2):
                        hs = slice(h * NB, (h + 1) * NB)
                        nc.scalar.activation(
                            et[:, hs], ps[:, hs], Exp, scale=descale,
                            accum_out=rs[:, mc + h : mc + h + 1],
                        )
                        nc.vector.tensor_add(
                            colp[:, hs], efold[:, hs], et[:, hs]
                        )
                        nc.sync.dma_start(
                            colp_d.ap()[:, hs], colp[:, hs]
                        )
                else:
                    et = epool.tile([128, 2 * NB], F32)
                    nc.scalar.activation(
                        et[:], ps[:], Exp, scale=descale,
                        accum_out=rs[:, mc : mc + 1],
                    )
                    nc.vector.tensor_add(efold[:], efold[:], et[:])

            nc.sync.dma_start(rowsum_d.ap(), rs[:])

    nc.compile()
    return nc


def _get_nc():
    if "nc" not in _CACHE:
        _CACHE["nc"] = _build_nc()
    return _CACHE["nc"]


def _prep_inputs(pre_VF, pre_AF, back_VF, back_AF):
    """Normalize + quantize + relayout on host; returns per-core in_maps,
    host diag, and the host pre-cosine term."""
    import ml_dtypes

    V = np.asarray(back_VF, dtype=np.float64)
    A = np.asarray(back_AF, dtype=np.float64)
    Vn = V / np.sqrt((V * V).sum(-1, keepdims=True) + EPS)
    An = A / np.sqrt((A * A).sum(-1, keepdims=True) + EPS)
    diag = np.einsum("ij,ij->i", Vn, An)

    pV = np.asarray(pre_VF, dtype=np.float64)
    pA = np.asarray(pre_AF, dtype=np.float64)
    pVn = pV / np.sqrt((pV * pV).sum(-1, keepdims=True) + EPS)
    pAn = pA / np.sqrt((pA * pA).sum(-1, keepdims=True) + EPS)
    pre_cos = np.einsum("ij,ij->i", pVn, pAn)

    fp8 = ml_dtypes.float8_e4m3
    Vn8 = (Vn * FP8_SCALE).astype(fp8)
    An8 = (An * FP8_SCALE).astype(fp8)

    # vnT[p, mc*1024 + k2*256 + i*128 + dm] = Vn8_slab[mc*128 + dm, (2*k2+i)*128 + p]
    vnTs = []
    for rg in range(RG):
        sl = Vn8[rg * ROWS : (rg + 1) * ROWS]
        vnTs.append(
            np.ascontiguousarray(
                sl.reshape(MCH, 128, KD2, 2, 128)  # [mc, dm, k2, i, p]
                .transpose(4, 0, 2, 3, 1)          # [p, mc, k2, i, dm]
                .reshape(128, MCH * KD2 * 2 * 128)
            )
        )

    # anT[n, p, k2*2*NB + i*NB + c] = An8_slab[n*NB + c, (2*k2+i)*128 + p]
    anTs = []
    for cg in range(CG):
        sl = An8[cg * COLS : (cg + 1) * COLS]
        anTs.append(
            np.ascontiguousarray(
                sl.reshape(NBL, NB, KD2, 2, 128)   # [n, c, k2, i, p]
                .transpose(0, 4, 2, 3, 1)          # [n, p, k2, i, c]
                .reshape(NBL, 128, KCH * NB)
            )
        )

    in_maps = []
    for c in range(NCORES):
        rg, cg = c // CG, c % CG
        in_maps.append({"vnT": vnTs[rg], "anT": anTs[cg]})
    return in_maps, diag, pre_cos


def _assemble(outs, diag, pre_cos):
    """O(N) final reduction on host, f64."""
    rowsum = np.zeros(N, dtype=np.float64)
    colsum = np.zeros(N, dtype=np.float64)
    for c in range(NCORES):
        rg, cg = c // CG, c % CG
        # rowsum[p, mc]: row = rg*ROWS + mc*128 + p, partial over this cg;
        # the last chunk's sum is split across columns MCH-1 and MCH
        rsc = outs[c]["rowsum"].astype(np.float64)  # [128, MCH + 1]
        rsc = np.concatenate(
            [rsc[:, : MCH - 1], (rsc[:, MCH - 1] + rsc[:, MCH])[:, None]], axis=1
        )
        rowsum[rg * ROWS : (rg + 1) * ROWS] += rsc.T.reshape(ROWS)
        colsum[cg * COLS : (cg + 1) * COLS] += (
            outs[c]["colp"].astype(np.float64).sum(axis=0)
        )

    dE = np.exp(diag)
    pos = np.exp(diag - MARGIN)
    neg_V = rowsum - dE
    neg_A = colsum - dE
    L_V = np.log(pos / (pos + neg_V)).sum()
    L_A = np.log(pos / (pos + neg_A)).sum()
    L_pre = pre_cos.sum()

    loss = BALANCE * (-1.0 / BIAS) * (L_V + L_A) + (1.0 - BALANCE) * L_pre
    return np.array(loss, dtype=np.float32)


def kernel(pre_VF, pre_AF, back_VF, back_AF):
    global LAST_RESULT
    from concourse import bass_utils

    nc = _get_nc()
    in_maps, diag, pre_cos = _prep_inputs(pre_VF, pre_AF, back_VF, back_AF)
    res = bass_utils.run_bass_kernel_spmd(nc, in_maps, core_ids=list(range(NCORES)))
    LAST_RESULT = res
    return _assemble(res.results, diag, pre_cos)
